# revision 13
# baseline (speedup 1.0000x reference)
"""TRN2 Bass kernel for nn_DetailTransformer (moe_routing).

Sharding: data-parallel over batch B=8 across 8 NeuronCores (one batch row
per core), parameters replicated. Per core the full forward runs with all
activations resident in SBUF:

  - residual stream x kept as [T-partition, D-free] fp32 tiles
  - big matmuls in float32r (full PE rate at N>=256, ~13-bit mantissa)
  - attention scores / probabilities / V and FFN second matmul in bf16
  - Transformer-XL rel-shift via a diagonal-AP SBUF->SBUF DMA
    (per-partition sliding window), validated on HW
  - biases folded into psum evictions (per-partition scalars) or K=1
    ones-matmuls (free-dim rows)
"""
import numpy as np
from contextlib import ExitStack

import concourse.bass as bass
import concourse.bacc as bacc
import concourse.tile as tile
from concourse import mybir
from concourse.bass_utils import run_bass_kernel_spmd

F32 = mybir.dt.float32
F32R = mybir.dt.float32r
BF16 = mybir.dt.bfloat16
AF = mybir.ActivationFunctionType
ALU = mybir.AluOpType
AX = mybir.AxisListType

N_CORES = 8
B, T = 8, 512
D_MOTION, D_CONTACT, D_PHASE, D_MASK = 128, 4, 16, 1
D_MODEL, D_ENC, D_DEC, D_GATE = 512, 512, 512, 128
N_LAYER, N_HEAD, D_HEAD, D_PFF = 4, 8, 64, 2048
E = D_PHASE // 2
CTX = 10
D_IN = D_MOTION + D_MASK + D_PHASE          # 145
D_OUT = D_MOTION + D_CONTACT                # 132
D_OUT_PAD = 256                             # ml_w2 padded N (fp32r full rate)
NT = T // 128                               # 4 token tiles
RW = 768                                    # bd window width (512 + 256)
RPAD = 1152                                 # padded rT columns

INV_SQRT_D = 1.0 / float(np.sqrt(D_HEAD))


def _f32r(ap):
    return ap.bitcast(F32R)


def build_forward(nc, tc, ctx, alphas, io, stages="full"):
    a_enc1, a_enc2, a_rp, a_pd, a_g1, a_g2, a_ml0, a_ml1 = alphas
    sync, act, dve, pe = nc.sync, nc.scalar, nc.vector, nc.tensor

    const = ctx.enter_context(tc.tile_pool(name="const", bufs=1))
    persist = ctx.enter_context(tc.tile_pool(name="persist", bufs=1))
    wq_p = ctx.enter_context(tc.tile_pool(name="wq_p", bufs=7))
    w512 = ctx.enter_context(tc.tile_pool(name="w512", bufs=6))
    wf1_p = ctx.enter_context(tc.tile_pool(name="wf1_p", bufs=7))
    wf2_p = ctx.enter_context(tc.tile_pool(name="wf2_p", bufs=16))
    wml_p = ctx.enter_context(tc.tile_pool(name="wml_p", bufs=5))
    wonce = ctx.enter_context(tc.tile_pool(name="wonce", bufs=1))
    colb = ctx.enter_context(tc.tile_pool(name="colb", bufs=14))
    rowb = ctx.enter_context(tc.tile_pool(name="rowb", bufs=2))
    atr = ctx.enter_context(tc.tile_pool(name="atr", bufs=2))
    hT_p = ctx.enter_context(tc.tile_pool(name="hT_p", bufs=1))
    qkv_p = ctx.enter_context(tc.tile_pool(name="qkv_p", bufs=1))
    oT_p = ctx.enter_context(tc.tile_pool(name="oT_p", bufs=1))
    a1_p = ctx.enter_context(tc.tile_pool(name="a1_p", bufs=1))
    zn_p = ctx.enter_context(tc.tile_pool(name="zn_p", bufs=1))
    acc_p = ctx.enter_context(tc.tile_pool(name="acc_p", bufs=1))
    ln_p = ctx.enter_context(tc.tile_pool(name="ln_p", bufs=2))
    st_p = ctx.enter_context(tc.tile_pool(name="st_p", bufs=4))
    misc = ctx.enter_context(tc.tile_pool(name="misc", bufs=1))
    psA = ctx.enter_context(tc.tile_pool(name="psA", bufs=2, space="PSUM"))
    psBD = ctx.enter_context(tc.tile_pool(name="psBD", bufs=1, space="PSUM"))
    psTR = ctx.enter_context(tc.tile_pool(name="psTR", bufs=2, space="PSUM"))
    psS = ctx.enter_context(tc.tile_pool(name="psS", bufs=2, space="PSUM"))

    # constants
    id_r = const.tile([128, 128], F32R, tag="id_r")
    id_b = const.tile([128, 128], BF16, tag="id_b")
    ones_r = const.tile([1, 128], F32R, tag="ones_r")
    eps_t = const.tile([128, 1], F32, tag="eps_t")
    sync.dma_start(id_r[:], _f32r(io["id128"][:]))
    sync.dma_start(id_b[:], io["id128b"][:])
    sync.dma_start(ones_r[:], _f32r(io["ones_row"][:]))
    dve.memset(eps_t[:], 1e-5)

    # ---------------- encoder input assembly ----------------
    xinT_a = persist.tile([128, T], F32R, tag="xinT_a")
    xinT_b = persist.tile([17, T], F32R, tag="xinT_b")
    phase_sb = []
    sync.dma_start(xinT_b[16:17, :], _f32r(io["dm_row"][:]))
    for m in range(NT):
        mo = misc.tile([128, 128], F32R, tag="mo_in")
        sync.dma_start(mo[:], _f32r(io["motion"][m * 128:(m + 1) * 128, :]))
        pmo = psA.tile([128, 128], F32, tag="pA")
        pe.transpose(_f32r(pmo[:]), mo[:], id_r[:])
        dve.tensor_copy(xinT_a[:, m * 128:(m + 1) * 128], pmo[:])

        phs = persist.tile([128, 16], F32R, tag=f"phase_{m}")
        sync.dma_start(phs[:], _f32r(io["phase"][m * 128:(m + 1) * 128, :]))
        phase_sb.append(phs)
        pph = psS.tile([16, 128], F32, tag="pS")
        pe.transpose(_f32r(pph[:]), phs[:], id_r[:])
        dve.tensor_copy(xinT_b[0:16, m * 128:(m + 1) * 128], pph[:])

    # ---------------- encoder MLP ----------------
    ew1a = wonce.tile([128, D_ENC], F32R, tag="ew1a")
    ew1b = wonce.tile([17, D_ENC], F32R, tag="ew1b")
    sync.dma_start(ew1a[:], _f32r(io["enc_w1"][0:128, :]))
    sync.dma_start(ew1b[0:16, :], _f32r(io["enc_w1"][129:145, :]))
    sync.dma_start(ew1b[16:17, :], _f32r(io["enc_w1"][128:129, :]))
    e1T = []
    for m in range(NT):
        p = psA.tile([128, T], F32, tag="pA")
        pe.matmul(p[:], ew1a[:, m * 128:(m + 1) * 128], xinT_a[:], start=True, stop=False)
        pe.matmul(p[:], ew1b[:, m * 128:(m + 1) * 128], xinT_b[:], start=False, stop=True)
        ebm = colb.tile([128, 1], F32, tag="colbias")
        sync.dma_start(ebm[:], io["enc_b1"][m * 128:(m + 1) * 128, :])
        t = acc_p.tile([128, T], F32R, tag=f"acc_{m}")
        act.activation(t[:], p[:], AF.Prelu, bias=ebm[:], alpha=a_enc1)
        e1T.append(t)

    ew2 = [w512.tile([128, D_MODEL], F32R, tag="w512", name=f"ew2_{k}") for k in range(4)]
    for k in range(4):
        sync.dma_start(ew2[k][:], _f32r(io["enc_w2"][k * 128:(k + 1) * 128, :]))
    eb2r = rowb.tile([1, D_MODEL], F32R, tag="rowbias")
    sync.dma_start(eb2r[:], _f32r(io["enc_b2r"][:]))
    x = []
    for m in range(NT):
        p = psA.tile([128, D_MODEL], F32, tag="pA")
        for k in range(4):
            pe.matmul(p[:], e1T[k][:, m * 128:(m + 1) * 128], ew2[k][:],
                      start=(k == 0), stop=False)
        pe.matmul(p[:], ones_r[:], eb2r[:], start=False, stop=True)
        xt = persist.tile([128, D_MODEL], F32, tag=f"x_{m}")
        act.activation(xt[:], p[:], AF.Prelu, alpha=a_enc2)
        x.append(xt)

    # ---------------- relative position embeddings ----------------
    posT = wonce.tile([1, 2 * T], F32R, tag="posT")
    sync.dma_start(posT[:], _f32r(io["posT"][:]))
    rw1 = wonce.tile([1, D_HEAD], F32R, tag="rw1")
    sync.dma_start(rw1[:], _f32r(io["rp_w1"][:]))
    rb1 = wonce.tile([64, 1], F32, tag="rb1")
    sync.dma_start(rb1[:], io["rp_b1"][:])
    rw2 = wonce.tile([64, D_HEAD], F32R, tag="rw2")
    sync.dma_start(rw2[:], _f32r(io["rp_w2"][:]))
    rb2 = wonce.tile([64, 1], F32, tag="rb2")
    sync.dma_start(rb2[:], io["rp_b2"][:])

    r1T = wonce.tile([64, 2 * T], F32R, tag="r1T")
    rT_pad = persist.tile([128, RPAD], BF16, tag="rT_pad")
    for c0 in (0, 512):
        p = psS.tile([64, 512], F32, tag="pS")
        pe.matmul(p[:], rw1[:], posT[:, c0:c0 + 512], start=True, stop=True)
        act.activation(r1T[:, c0:c0 + 512], p[:], AF.Prelu, bias=rb1[:], alpha=a_rp)
    for c0 in (0, 512):
        p = psS.tile([64, 512], F32, tag="pS")
        pe.matmul(p[:], rw2[:], r1T[:, c0:c0 + 512], start=True, stop=True)
        dve.tensor_scalar(rT_pad[0:64, c0:c0 + 512], p[:], rb2[:], INV_SQRT_D,
                          op0=ALU.add, op1=ALU.mult)
    dve.tensor_scalar(rT_pad[0:64, 1023:RPAD], rT_pad[0:64, 0:RPAD - 1023], 0.0, None,
                      op0=ALU.mult)
    dve.tensor_copy(rT_pad[64:128, :], rT_pad[0:64, :])

    def dump_x(x):
        for m in range(NT):
            sync.dma_start(io["out"][m * 128:(m + 1) * 128, 0:D_OUT],
                           x[m][:, 0:D_OUT])
            sync.dma_start(io["out"][m * 128:(m + 1) * 128, D_OUT:148],
                           x[m][:, D_OUT:148])
    if stages == "enc":
        dump_x(x)
        return

    # ---------------- layers ----------------
    def layernorm_to_T(xtiles, g_dram, b_dram):
        gcol = [colb.tile([128, 1], F32, tag="colbias", name=f"gcol{k}") for k in range(4)]
        bcol = [colb.tile([128, 1], F32, tag="colbias", name=f"bcolv{k}") for k in range(4)]
        for k in range(4):
            sync.dma_start(gcol[k][:], g_dram[k * 128:(k + 1) * 128, :])
            sync.dma_start(bcol[k][:], b_dram[k * 128:(k + 1) * 128, :])
        hT = [hT_p.tile([128, T], F32R, tag=f"hT_{k}", name=f"hT_{k}") for k in range(4)]
        for m in range(NT):
            xt = xtiles[m]
            s = st_p.tile([128, 1], F32, tag="ln_s")
            dve.tensor_reduce(s[:], xt[:], axis=AX.X, op=ALU.add)
            scr = ln_p.tile([128, D_MODEL], BF16, tag="ln_scr")
            ss = st_p.tile([128, 1], F32, tag="ln_ss")
            act.activation(scr[:], xt[:], AF.Square, accum_out=ss[:])
            mean = st_p.tile([128, 1], F32, tag="ln_m")
            dve.tensor_scalar(mean[:], s[:], 1.0 / D_MODEL, None, op0=ALU.mult)
            msq = st_p.tile([128, 1], F32, tag="ln_msq")
            dve.tensor_tensor(msq[:], mean[:], mean[:], op=ALU.mult)
            var = st_p.tile([128, 1], F32, tag="ln_v")
            dve.scalar_tensor_tensor(var[:], ss[:], 1.0 / D_MODEL, msq[:],
                                     op0=ALU.mult, op1=ALU.subtract)
            std = st_p.tile([128, 1], F32, tag="ln_std")
            act.activation(std[:], var[:], AF.Sqrt, bias=eps_t[:])
            rstd = st_p.tile([128, 1], F32, tag="ln_r")
            dve.reciprocal(rstd[:], std[:])
            t1 = ln_p.tile([128, D_MODEL], F32R, tag="ln_t1")
            dve.tensor_scalar(t1[:], xt[:], mean[:], rstd[:],
                              op0=ALU.subtract, op1=ALU.mult)
            for k in range(4):
                p = psA.tile([128, 128], F32, tag="pA")
                pe.transpose(_f32r(p[:]), t1[:, k * 128:(k + 1) * 128], id_r[:])
                dve.tensor_scalar(hT[k][:, m * 128:(m + 1) * 128], p[:],
                                  gcol[k][:], bcol[k][:], op0=ALU.mult, op1=ALU.add)
        return hT

    n_layer_run = N_LAYER if stages in ("full",) else 1
    for li in range(n_layer_run):
        # ===== attention =====
        hT = layernorm_to_T(x, io["att_ln_g"][li], io["att_ln_b"][li])

        qT, kT, vbf = [], [], []
        wq_cur = None
        for m in range(12):
            if m % 4 == 0:
                g = m // 4
                wq_cur = [wq_p.tile([128, 512], F32R, tag="wqkv", name=f"wq{m}_{k}") for k in range(4)]
                for k in range(4):
                    sync.dma_start(wq_cur[k][:], _f32r(
                        io["att_qkv_w"][li][k * 128:(k + 1) * 128,
                                            g * 512:(g + 1) * 512]))
            p = psA.tile([128, T], F32, tag="pA")
            for k in range(4):
                pe.matmul(p[:], wq_cur[k][:, (m % 4) * 128:(m % 4 + 1) * 128],
                          hT[k][:], start=(k == 0), stop=(k == 3))
            bcol = colb.tile([128, 1], F32, tag="colbias")
            sync.dma_start(bcol[:], io["att_qkv_b"][li][m * 128:(m + 1) * 128, :])
            if m < 4:       # q -> bf16
                t = qkv_p.tile([128, T], BF16, tag=f"qT_{m}")
                dve.tensor_scalar(t[:], p[:], bcol[:], None, op0=ALU.add)
                qT.append(t)
            elif m < 8:     # k -> bf16, pre-scaled by 1/sqrt(d)
                t = qkv_p.tile([128, T], BF16, tag=f"kT_{m - 4}")
                dve.tensor_scalar(t[:], p[:], bcol[:], INV_SQRT_D,
                                  op0=ALU.add, op1=ALU.mult)
                kT.append(t)
            else:           # v -> bf16
                t = qkv_p.tile([128, T], BF16, tag=f"vbf_{m - 8}")
                dve.tensor_scalar(t[:], p[:], bcol[:], None, op0=ALU.add)
                vbf.append(t)

        v_jd = [qkv_p.tile([128, N_HEAD * D_HEAD], BF16, tag=f"vjd_{jb}", name=f"vjd_{jb}")
                for jb in range(NT)]
        for h in range(N_HEAD):
            vt = vbf[h // 2]
            p0 = (h % 2) * 64
            for jb in range(NT):
                pv = psTR.tile([128, 64], BF16, tag="pTR")
                pe.transpose(pv[:], vt[p0:p0 + 64, jb * 128:(jb + 1) * 128],
                             id_b[p0:p0 + 64, p0:p0 + 64])
                dve.tensor_copy(v_jd[jb][:, h * 64:(h + 1) * 64], pv[:])

        oT = [oT_p.tile([128, T], BF16, tag=f"oT_{t}", name=f"oT_{t}") for t in range(NT)]
        for h in range(N_HEAD):
            p0 = (h % 2) * 64
            for ti in range(NT):
                qs = qT[h // 2][p0:p0 + 64, ti * 128:(ti + 1) * 128]
                pac = psA.tile([128, T], F32, tag="pA")
                pe.matmul(pac[:], qs, kT[h // 2][p0:p0 + 64, :], start=True, stop=True)
                w0 = 384 - 128 * ti
                pbd = psBD.tile([128, RW], F32, tag="pBD")
                pe.matmul(pbd[:, 0:512], qs, rT_pad[p0:p0 + 64, w0:w0 + 512],
                          start=True, stop=True)
                pe.matmul(pbd[:, 512:RW], qs, rT_pad[p0:p0 + 64, w0 + 512:w0 + RW],
                          start=True, stop=True)
                bd_sb = atr.tile([128, RW], BF16, tag="bd_sb")
                dve.tensor_copy(bd_sb[:], pbd[:])
                shift = atr.tile([128, T], BF16, tag="shift")
                diag = bass.AP(tensor=bd_sb[:].tensor,
                               offset=bd_sb[:].offset + 127,
                               ap=[[RW - 1, 128], [1, T]])
                act.dma_start(shift[:], diag)
                scores = atr.tile([128, T], BF16, tag="scores")
                dve.tensor_tensor(scores[:], pac[:], shift[:], op=ALU.add)
                expb = atr.tile([128, T], BF16, tag="expb")
                sums = st_p.tile([128, 1], F32, tag="sums")
                act.activation(expb[:], scores[:], AF.Exp, accum_out=sums[:])
                recip = st_p.tile([128, 1], F32, tag="recip")
                dve.reciprocal(recip[:], sums[:])
                attn = atr.tile([128, T], BF16, tag="attn")
                dve.tensor_scalar(attn[:], expb[:], recip[:], None, op0=ALU.mult)
                pat = psTR.tile([128, 512], BF16, tag="pTR")
                for jb in range(NT):
                    pe.transpose(pat[:, jb * 128:(jb + 1) * 128],
                                 attn[:, jb * 128:(jb + 1) * 128], id_b[:])
                attnT = atr.tile([128, T], BF16, tag="attnT")
                act.activation(attnT[:], pat[:], AF.Copy)
                po = psS.tile([64, 128], F32, tag="pS")
                for jb in range(NT):
                    pe.matmul(po[:], v_jd[jb][:, h * 64:(h + 1) * 64],
                              attnT[:, jb * 128:(jb + 1) * 128],
                              start=(jb == 0), stop=(jb == 3))
                dve.tensor_copy(oT[h // 2][p0:p0 + 64, ti * 128:(ti + 1) * 128],
                                po[:])

        wo = [w512.tile([128, D_MODEL], BF16, tag="w512b", name=f"wo_{k}") for k in range(4)]
        for k in range(4):
            sync.dma_start(wo[k][:], io["att_o_wb"][li][k * 128:(k + 1) * 128, :])
        obr = rowb.tile([1, D_MODEL], F32R, tag="rowbias")
        sync.dma_start(obr[:], _f32r(io["att_o_br"][li]))
        for m in range(NT):
            p = psA.tile([128, D_MODEL], F32, tag="pA")
            for k in range(4):
                pe.matmul(p[:], oT[k][:, m * 128:(m + 1) * 128], wo[k][:],
                          start=(k == 0), stop=False)
            pe.matmul(p[:], ones_r[:], obr[:], start=False, stop=True)
            dve.tensor_tensor(x[m][:], p[:], x[m][:], op=ALU.add)

        if stages == "att1":
            dump_x(x)
            return
        # ===== feed-forward =====
        h2T = layernorm_to_T(x, io["pff_ln_g"][li], io["pff_ln_b"][li])
        a1T = []
        wf_cur = None
        for m in range(16):
            if m % 4 == 0:
                g = m // 4
                wf_cur = [wf1_p.tile([128, 512], F32R, tag="wff1", name=f"wf{m}_{k}") for k in range(4)]
                for k in range(4):
                    sync.dma_start(wf_cur[k][:], _f32r(
                        io["pff_w1"][li][k * 128:(k + 1) * 128,
                                         g * 512:(g + 1) * 512]))
            p = psA.tile([128, T], F32, tag="pA")
            for k in range(4):
                pe.matmul(p[:], wf_cur[k][:, (m % 4) * 128:(m % 4 + 1) * 128],
                          h2T[k][:], start=(k == 0), stop=(k == 3))
            bcol = colb.tile([128, 1], F32, tag="colbias")
            sync.dma_start(bcol[:], io["pff_b1"][li][m * 128:(m + 1) * 128, :])
            t = a1_p.tile([128, T], BF16, tag=f"a1T_{m}")
            act.activation(t[:], p[:], AF.Relu, bias=bcol[:])
            a1T.append(t)
        wf2 = [wf2_p.tile([128, D_MODEL], BF16, tag="wff2", name=f"wf2_{k}") for k in range(16)]
        for k in range(16):
            sync.dma_start(wf2[k][:], io["pff_w2b"][li][k * 128:(k + 1) * 128, :])
        fbr = rowb.tile([1, D_MODEL], F32R, tag="rowbias")
        sync.dma_start(fbr[:], _f32r(io["pff_b2r"][li]))
        for m in range(NT):
            p = psA.tile([128, D_MODEL], F32, tag="pA")
            for k in range(16):
                pe.matmul(p[:], a1T[k][:, m * 128:(m + 1) * 128], wf2[k][:],
                          start=(k == 0), stop=False)
            pe.matmul(p[:], ones_r[:], fbr[:], start=False, stop=True)
            dve.tensor_tensor(x[m][:], p[:], x[m][:], op=ALU.add)

    if stages == "ffn1":
        dump_x(x)
        return
    # ---------------- final layernorm ----------------
    xfT = layernorm_to_T(x, io["final_ln_g"], io["final_ln_b"])

    # ---------------- phase decoder ----------------
    wp1 = [w512.tile([128, D_DEC], F32R, tag="w512", name=f"wp1_{k}") for k in range(4)]
    for k in range(4):
        sync.dma_start(wp1[k][:], _f32r(io["pd_w1"][k * 128:(k + 1) * 128, :]))
    p1T = []
    for m in range(NT):
        p = psA.tile([128, T], F32, tag="pA")
        for k in range(4):
            pe.matmul(p[:], wp1[k][:, m * 128:(m + 1) * 128], xfT[k][:],
                      start=(k == 0), stop=(k == 3))
        bcol = colb.tile([128, 1], F32, tag="colbias")
        sync.dma_start(bcol[:], io["pd_b1"][m * 128:(m + 1) * 128, :])
        t = zn_p.tile([128, T], F32R, tag=f"znT_{m}", name=f"p1T_{m}")
        act.activation(t[:], p[:], AF.Prelu, bias=bcol[:], alpha=a_pd)
        p1T.append(t)
    wp2 = [w512.tile([128, D_PHASE], F32R, tag="w512", name=f"wp2_{k}") for k in range(4)]
    for k in range(4):
        sync.dma_start(wp2[k][:], _f32r(io["pd_w2"][k * 128:(k + 1) * 128, :]))
    pb2r = rowb.tile([1, D_PHASE], F32R, tag="pb2r")
    sync.dma_start(pb2r[:], _f32r(io["pd_b2r"][:]))
    ph = []
    phT = persist.tile([16, T], F32R, tag="phT")
    for m in range(NT):
        p = psS.tile([128, 16], F32, tag="pS")
        for k in range(4):
            pe.matmul(p[:], p1T[k][:, m * 128:(m + 1) * 128], wp2[k][:],
                      start=(k == 0), stop=False)
        pe.matmul(p[:], ones_r[:], pb2r[:], start=False, stop=True)
        pht = persist.tile([128, D_PHASE], F32R, tag=f"ph_{m}")
        dve.tensor_tensor(pht[:], p[:], phase_sb[m][:].bitcast(F32), op=ALU.add)
        ph.append(pht)
        sync.dma_start(io["out"][m * 128:(m + 1) * 128, D_OUT:D_OUT + D_PHASE],
                       pht[:].bitcast(F32))
        pt = psS.tile([16, 128], F32, tag="pS")
        pe.transpose(_f32r(pt[:]), pht[:], id_r[:])
        dve.tensor_copy(phT[:, m * 128:(m + 1) * 128], pt[:])

    # ---------------- gating network ----------------
    gw1 = wonce.tile([16, D_GATE], F32R, tag="gw1")
    sync.dma_start(gw1[:], _f32r(io["g_w1"][:]))
    gb1 = colb.tile([128, 1], F32, tag="colbias")
    sync.dma_start(gb1[:], io["g_b1"][:])
    pg = psA.tile([128, T], F32, tag="pA")
    pe.matmul(pg[:], gw1[:], phT[:], start=True, stop=True)
    g1T = misc.tile([128, T], F32R, tag="g1T")
    act.activation(g1T[:], pg[:], AF.Prelu, bias=gb1[:], alpha=a_g1)

    gw2 = wonce.tile([128, D_GATE], F32R, tag="gw2")
    sync.dma_start(gw2[:], _f32r(io["g_w2"][:]))
    gb2 = colb.tile([128, 1], F32, tag="colbias")
    sync.dma_start(gb2[:], io["g_b2"][:])
    pg2 = psA.tile([128, T], F32, tag="pA")
    pe.matmul(pg2[:], gw2[:], g1T[:], start=True, stop=True)
    g2T = misc.tile([128, T], F32R, tag="g2T")
    act.activation(g2T[:], pg2[:], AF.Prelu, bias=gb2[:], alpha=a_g2)

    gw3 = wonce.tile([128, E], F32R, tag="gw3")
    sync.dma_start(gw3[:], _f32r(io["g_w3"][:]))
    gb3 = wonce.tile([E, 1], F32, tag="gb3")
    sync.dma_start(gb3[:], io["g_b3"][:])
    pg3 = psS.tile([E, T], F32, tag="pS")
    pe.matmul(pg3[:], gw3[:], g2T[:], start=True, stop=True)
    lT = misc.tile([E, T], F32R, tag="lT")
    dve.tensor_scalar(lT[:], pg3[:], gb3[:], None, op0=ALU.add)

    gw = []
    gwT = persist.tile([E, T], F32R, tag="gwT")
    for m in range(NT):
        p = psS.tile([128, E], F32, tag="pS")
        pe.transpose(_f32r(p[:]), lT[:, m * 128:(m + 1) * 128], id_r[0:E, 0:E])
        ge = misc.tile([128, E], F32, tag="ge")
        gs = st_p.tile([128, 1], F32, tag="gs")
        act.activation(ge[:], p[:], AF.Exp, accum_out=gs[:])
        gr = st_p.tile([128, 1], F32, tag="gr")
        dve.reciprocal(gr[:], gs[:])
        gwm = persist.tile([128, E], F32R, tag=f"gw_{m}")
        dve.tensor_scalar(gwm[:], ge[:], gr[:], None, op0=ALU.mult)
        gw.append(gwm)
        pt = psS.tile([E, 128], F32, tag="pS")
        pe.transpose(_f32r(pt[:]), gwm[:], id_r[:])
        dve.tensor_copy(gwT[:, m * 128:(m + 1) * 128], pt[:])

    # ---------------- MoE blends ----------------
    zT = xfT
    for L, (wname, bname, alpha, NOUT) in enumerate((
            ("ml_w0", "ml_b0", a_ml0, D_DEC),
            ("ml_w1", "ml_b1", a_ml1, D_DEC),
            ("ml_w2p", "ml_b2p", None, D_OUT_PAD))):
        mlb = wonce.tile([E, NOUT], F32R, tag=f"mlb_{L}")
        sync.dma_start(mlb[:], _f32r(io[bname][:]))
        acc = [acc_p.tile([128, NOUT], F32R, tag=f"acc_{m}", name=f"accL{L}_{m}") for m in range(NT)]
        for m in range(NT):
            p = psA.tile([128, NOUT], F32, tag="pA")
            pe.matmul(p[:], gwT[:, m * 128:(m + 1) * 128], mlb[:],
                      start=True, stop=True)
            dve.tensor_copy(acc[m][:], p[:])
        for e in range(E):
            we = [wml_p.tile([128, NOUT], F32R, tag="wml", name=f"we{L}_{e}_{k}") for k in range(4)]
            for k in range(4):
                sync.dma_start(we[k][:], _f32r(io[wname][e][k * 128:(k + 1) * 128, :]))
            for m in range(NT):
                p = psA.tile([128, NOUT], F32, tag="pA")
                for k in range(4):
                    pe.matmul(p[:], zT[k][:, m * 128:(m + 1) * 128], we[k][:],
                              start=(k == 0), stop=(k == 3))
                dve.scalar_tensor_tensor(acc[m][:], p[:],
                                         gw[m][:, e:e + 1].bitcast(F32),
                                         acc[m][:].bitcast(F32),
                                         op0=ALU.mult, op1=ALU.add)
        if L < 2:
            znT = [zn_p.tile([128, T], F32R, tag=f"znT_{k}", name=f"znT{L}_{k}") for k in range(4)]
            for m in range(NT):
                for k in range(4):
                    p = psA.tile([128, 128], F32, tag="pA")
                    pe.transpose(_f32r(p[:]), acc[m][:, k * 128:(k + 1) * 128],
                                 id_r[:])
                    act.activation(znT[k][:, m * 128:(m + 1) * 128], p[:],
                                   AF.Prelu, alpha=alpha)
            zT = znT
        else:
            for m in range(NT):
                sync.dma_start(io["out"][m * 128:(m + 1) * 128, 0:D_OUT],
                               acc[m][:, 0:D_OUT].bitcast(F32))


def build_program(alphas, n_reps=1, stages="full"):
    nc = bacc.Bacc("TRN2", target_bir_lowering=False, debug=False)
    io = {}

    def inp(name, shape, dt=F32):
        io[name] = nc.dram_tensor(name, list(shape), dt, kind="ExternalInput").ap()

    inp("motion", (T, D_MOTION))
    inp("phase", (T, D_PHASE))
    inp("dm_row", (1, T))
    inp("posT", (1, 2 * T))
    inp("ones_row", (1, 128))
    inp("id128", (128, 128))
    inp("id128b", (128, 128), BF16)

    inp("enc_w1", (D_IN, D_ENC)); inp("enc_b1", (D_ENC, 1))
    inp("enc_w2", (D_ENC, D_MODEL)); inp("enc_b2r", (1, D_MODEL))
    inp("rp_w1", (1, D_HEAD)); inp("rp_b1", (D_HEAD, 1))
    inp("rp_w2", (D_HEAD, D_HEAD)); inp("rp_b2", (D_HEAD, 1))
    inp("att_qkv_w", (N_LAYER, D_MODEL, 3 * D_MODEL))
    inp("att_qkv_b", (N_LAYER, 3 * D_MODEL, 1))
    inp("att_o_wb", (N_LAYER, D_MODEL, D_MODEL), BF16)
    inp("att_o_br", (N_LAYER, 1, D_MODEL))
    inp("att_ln_g", (N_LAYER, D_MODEL, 1)); inp("att_ln_b", (N_LAYER, D_MODEL, 1))
    inp("pff_w1", (N_LAYER, D_MODEL, D_PFF)); inp("pff_b1", (N_LAYER, D_PFF, 1))
    inp("pff_w2b", (N_LAYER, D_PFF, D_MODEL), BF16)
    inp("pff_b2r", (N_LAYER, 1, D_MODEL))
    inp("pff_ln_g", (N_LAYER, D_MODEL, 1)); inp("pff_ln_b", (N_LAYER, D_MODEL, 1))
    inp("final_ln_g", (D_MODEL, 1)); inp("final_ln_b", (D_MODEL, 1))
    inp("pd_w1", (D_MODEL, D_DEC)); inp("pd_b1", (D_DEC, 1))
    inp("pd_w2", (D_DEC, D_PHASE)); inp("pd_b2r", (1, D_PHASE))
    inp("g_w1", (D_PHASE, D_GATE)); inp("g_b1", (D_GATE, 1))
    inp("g_w2", (D_GATE, D_GATE)); inp("g_b2", (D_GATE, 1))
    inp("g_w3", (D_GATE, E)); inp("g_b3", (E, 1))
    inp("ml_w0", (E, D_MODEL, D_DEC)); inp("ml_b0", (E, D_DEC))
    inp("ml_w1", (E, D_DEC, D_DEC)); inp("ml_b1", (E, D_DEC))
    inp("ml_w2p", (E, D_DEC, D_OUT_PAD)); inp("ml_b2p", (E, D_OUT_PAD))

    io["out"] = nc.dram_tensor("out", [T, D_OUT + D_PHASE], F32,
                               kind="ExternalOutput").ap()

    with tile.TileContext(nc) as tc, ExitStack() as ctx:
        if n_reps == 1:
            build_forward(nc, tc, ctx, alphas, io, stages)
        else:
            with tc.For_i(0, n_reps, 1):
                with ExitStack() as inner:
                    build_forward(nc, tc, inner, alphas, io, stages)
    nc.compile()
    return nc


def prepare_weight_maps(params):
    import ml_dtypes
    p = {k: np.asarray(v) for k, v in params.items()}
    f = np.float32
    w = {}
    w["enc_w1"] = p["enc_w1"].astype(f)
    w["enc_b1"] = p["enc_b1"].astype(f).reshape(-1, 1)
    w["enc_w2"] = p["enc_w2"].astype(f)
    w["enc_b2r"] = p["enc_b2"].astype(f).reshape(1, -1)
    w["rp_w1"] = p["rp_w1"].astype(f).reshape(1, D_HEAD)
    w["rp_b1"] = p["rp_b1"].astype(f).reshape(-1, 1)
    w["rp_w2"] = p["rp_w2"].astype(f)
    w["rp_b2"] = p["rp_b2"].astype(f).reshape(-1, 1)
    w["att_qkv_w"] = p["att_qkv_w"].astype(f)
    w["att_qkv_b"] = p["att_qkv_b"].astype(f).reshape(N_LAYER, -1, 1)
    w["att_o_wb"] = p["att_o_w"].astype(ml_dtypes.bfloat16)
    w["att_o_br"] = p["att_o_b"].astype(f).reshape(N_LAYER, 1, -1)
    w["att_ln_g"] = p["att_ln_g"].astype(f).reshape(N_LAYER, -1, 1)
    w["att_ln_b"] = p["att_ln_b"].astype(f).reshape(N_LAYER, -1, 1)
    w["pff_w1"] = p["pff_w1"].astype(f)
    w["pff_b1"] = p["pff_b1"].astype(f).reshape(N_LAYER, -1, 1)
    w["pff_w2b"] = p["pff_w2"].astype(ml_dtypes.bfloat16)
    w["pff_b2r"] = p["pff_b2"].astype(f).reshape(N_LAYER, 1, -1)
    w["pff_ln_g"] = p["pff_ln_g"].astype(f).reshape(N_LAYER, -1, 1)
    w["pff_ln_b"] = p["pff_ln_b"].astype(f).reshape(N_LAYER, -1, 1)
    w["final_ln_g"] = p["final_ln_g"].astype(f).reshape(-1, 1)
    w["final_ln_b"] = p["final_ln_b"].astype(f).reshape(-1, 1)
    w["pd_w1"] = p["pd_w1"].astype(f)
    w["pd_b1"] = p["pd_b1"].astype(f).reshape(-1, 1)
    w["pd_w2"] = p["pd_w2"].astype(f)
    w["pd_b2r"] = p["pd_b2"].astype(f).reshape(1, -1)
    w["g_w1"] = p["g_w1"].astype(f)
    w["g_b1"] = p["g_b1"].astype(f).reshape(-1, 1)
    w["g_w2"] = p["g_w2"].astype(f)
    w["g_b2"] = p["g_b2"].astype(f).reshape(-1, 1)
    w["g_w3"] = p["g_w3"].astype(f)
    w["g_b3"] = p["g_b3"].astype(f).reshape(-1, 1)
    w["ml_w0"] = p["ml_w0"].astype(f)
    w["ml_b0"] = p["ml_b0"].astype(f)
    w["ml_w1"] = p["ml_w1"].astype(f)
    w["ml_b1"] = p["ml_b1"].astype(f)
    w2p = np.zeros((E, D_DEC, D_OUT_PAD), f)
    w2p[:, :, :D_OUT] = p["ml_w2"].astype(f)
    w["ml_w2p"] = w2p
    b2p = np.zeros((E, D_OUT_PAD), f)
    b2p[:, :D_OUT] = p["ml_b2"].astype(f)
    w["ml_b2p"] = b2p
    return w


def extract_alphas(params):
    return tuple(float(np.asarray(params[k])) for k in
                 ("enc_a1", "enc_a2", "rp_a", "pd_a", "g_a1", "g_a2",
                  "ml_a0", "ml_a1"))


_CACHE = {}


def get_program(alphas, n_reps=1, stages="full"):
    key = (alphas, n_reps, stages)
    if key not in _CACHE:
        _CACHE[key] = build_program(alphas, n_reps, stages)
    return _CACHE[key]


def make_in_maps(motion, phase, midway_targets, params):
    import ml_dtypes
    motion = np.asarray(motion, np.float32)
    phase = np.asarray(phase, np.float32)
    mt = np.asarray(midway_targets).astype(np.int64)

    dm = np.zeros(T, np.float32)
    dm[:CTX] = 1.0
    dm[-1] = 1.0
    dm[mt] = 1.0

    shared = prepare_weight_maps(params)
    shared["dm_row"] = dm.reshape(1, T)
    shared["posT"] = np.arange(-T + 1, T + 1, dtype=np.float32).reshape(1, 2 * T)
    shared["ones_row"] = np.ones((1, 128), np.float32)
    shared["id128"] = np.eye(128, dtype=np.float32)
    shared["id128b"] = np.eye(128, dtype=ml_dtypes.bfloat16)

    in_maps = []
    for c in range(N_CORES):
        m = dict(shared)
        m["motion"] = motion[c]
        m["phase"] = phase[c]
        in_maps.append(m)
    return in_maps


def kernel(motion, phase, midway_targets, params):
    alphas = extract_alphas(params)
    nc = get_program(alphas, 1)
    in_maps = make_in_maps(motion, phase, midway_targets, params)
    res = run_bass_kernel_spmd(nc, in_maps, list(range(N_CORES)))
    out = np.stack([res.results[c]["out"] for c in range(N_CORES)], axis=0)
    return out.astype(np.float32)


# revision 14
# speedup vs baseline: 8.9408x; 8.9408x over previous
"""TRN2 Bass kernel for nn_DetailTransformer (moe_routing).

Sharding: data-parallel over batch B=8 across 8 NeuronCores (one batch row
per core), parameters replicated. Per core the full forward runs with all
activations resident in SBUF:

  - residual stream x kept as [T-partition, D-free] fp32 tiles
  - big matmuls in float32r (full PE rate at N>=256, ~13-bit mantissa)
  - attention scores / probabilities / V and FFN second matmul in bf16
  - Transformer-XL rel-shift via a diagonal-AP SBUF->SBUF DMA
    (per-partition sliding window), validated on HW
  - biases folded into psum evictions (per-partition scalars) or K=1
    ones-matmuls (free-dim rows)
"""
import numpy as np
from contextlib import ExitStack

import concourse.bass as bass
import concourse.bacc as bacc
import concourse.tile as tile
from concourse import mybir
from concourse.bass_utils import run_bass_kernel_spmd

F32 = mybir.dt.float32
F32R = mybir.dt.float32r
BF16 = mybir.dt.bfloat16
AF = mybir.ActivationFunctionType
ALU = mybir.AluOpType
AX = mybir.AxisListType

N_CORES = 8
B, T = 8, 512
D_MOTION, D_CONTACT, D_PHASE, D_MASK = 128, 4, 16, 1
D_MODEL, D_ENC, D_DEC, D_GATE = 512, 512, 512, 128
N_LAYER, N_HEAD, D_HEAD, D_PFF = 4, 8, 64, 2048
E = D_PHASE // 2
CTX = 10
D_IN = D_MOTION + D_MASK + D_PHASE          # 145
D_OUT = D_MOTION + D_CONTACT                # 132
D_OUT_PAD = 256                             # ml_w2 padded N (fp32r full rate)
NT = T // 128                               # 4 token tiles
RW = 768                                    # bd window width (512 + 256)
RPAD = 1152                                 # padded rT columns

INV_SQRT_D = 1.0 / float(np.sqrt(D_HEAD))


def _f32r(ap):
    return ap.bitcast(F32R)


def build_forward(nc, tc, ctx, alphas, io, stages="full"):
    a_enc1, a_enc2, a_rp, a_pd, a_g1, a_g2, a_ml0, a_ml1 = alphas
    sync, act, dve, pe = nc.sync, nc.scalar, nc.vector, nc.tensor

    const = ctx.enter_context(tc.tile_pool(name="const", bufs=1))
    persist = ctx.enter_context(tc.tile_pool(name="persist", bufs=1))
    wq_p = ctx.enter_context(tc.tile_pool(name="wq_p", bufs=7))
    w512 = ctx.enter_context(tc.tile_pool(name="w512", bufs=6))
    wf1_p = ctx.enter_context(tc.tile_pool(name="wf1_p", bufs=7))
    wf2_p = ctx.enter_context(tc.tile_pool(name="wf2_p", bufs=16))
    wml_p = ctx.enter_context(tc.tile_pool(name="wml_p", bufs=5))
    wonce = ctx.enter_context(tc.tile_pool(name="wonce", bufs=1))
    colb = ctx.enter_context(tc.tile_pool(name="colb", bufs=14))
    rowb = ctx.enter_context(tc.tile_pool(name="rowb", bufs=2))
    atr = ctx.enter_context(tc.tile_pool(name="atr", bufs=2))
    hT_p = ctx.enter_context(tc.tile_pool(name="hT_p", bufs=1))
    qkv_p = ctx.enter_context(tc.tile_pool(name="qkv_p", bufs=1))
    oT_p = ctx.enter_context(tc.tile_pool(name="oT_p", bufs=1))
    a1_p = ctx.enter_context(tc.tile_pool(name="a1_p", bufs=1))
    zn_p = ctx.enter_context(tc.tile_pool(name="zn_p", bufs=1))
    acc_p = ctx.enter_context(tc.tile_pool(name="acc_p", bufs=1))
    ln_p = ctx.enter_context(tc.tile_pool(name="ln_p", bufs=2))
    st_p = ctx.enter_context(tc.tile_pool(name="st_p", bufs=4))
    misc = ctx.enter_context(tc.tile_pool(name="misc", bufs=1))
    psA = ctx.enter_context(tc.tile_pool(name="psA", bufs=2, space="PSUM"))
    psBD = ctx.enter_context(tc.tile_pool(name="psBD", bufs=1, space="PSUM"))
    psTR = ctx.enter_context(tc.tile_pool(name="psTR", bufs=2, space="PSUM"))
    psS = ctx.enter_context(tc.tile_pool(name="psS", bufs=2, space="PSUM"))

    # constants
    id_r = const.tile([128, 128], F32R, tag="id_r")
    id_b = const.tile([128, 128], BF16, tag="id_b")
    ones_r = const.tile([1, 128], F32R, tag="ones_r")
    eps_t = const.tile([128, 1], F32, tag="eps_t")
    sync.dma_start(id_r[:], _f32r(io["id128"][:]))
    sync.dma_start(id_b[:], io["id128b"][:])
    sync.dma_start(ones_r[:], _f32r(io["ones_row"][:]))
    dve.memset(eps_t[:], 1e-5)

    # ---------------- encoder input assembly ----------------
    xinT_a = persist.tile([128, T], F32R, tag="xinT_a")
    xinT_b = persist.tile([17, T], F32R, tag="xinT_b")
    phase_sb = []
    sync.dma_start(xinT_b[16:17, :], _f32r(io["dm_row"][:]))
    for m in range(NT):
        mo = misc.tile([128, 128], F32R, tag="mo_in")
        sync.dma_start(mo[:], _f32r(io["motion"][m * 128:(m + 1) * 128, :]))
        pmo = psA.tile([128, 128], F32, tag="pA")
        pe.transpose(_f32r(pmo[:]), mo[:], id_r[:])
        dve.tensor_copy(xinT_a[:, m * 128:(m + 1) * 128], pmo[:])

        phs = persist.tile([128, 16], F32R, tag=f"phase_{m}")
        sync.dma_start(phs[:], _f32r(io["phase"][m * 128:(m + 1) * 128, :]))
        phase_sb.append(phs)
        pph = psS.tile([16, 128], F32, tag="pS")
        pe.transpose(_f32r(pph[:]), phs[:], id_r[:])
        dve.tensor_copy(xinT_b[0:16, m * 128:(m + 1) * 128], pph[:])

    # ---------------- encoder MLP ----------------
    ew1a = wonce.tile([128, D_ENC], F32R, tag="ew1a")
    ew1b = wonce.tile([17, D_ENC], F32R, tag="ew1b")
    sync.dma_start(ew1a[:], _f32r(io["enc_w1"][0:128, :]))
    sync.dma_start(ew1b[0:16, :], _f32r(io["enc_w1"][129:145, :]))
    sync.dma_start(ew1b[16:17, :], _f32r(io["enc_w1"][128:129, :]))
    e1T = []
    for m in range(NT):
        p = psA.tile([128, T], F32, tag="pA")
        pe.matmul(p[:], ew1a[:, m * 128:(m + 1) * 128], xinT_a[:], start=True, stop=False)
        pe.matmul(p[:], ew1b[:, m * 128:(m + 1) * 128], xinT_b[:], start=False, stop=True)
        ebm = colb.tile([128, 1], F32, tag="colbias")
        sync.dma_start(ebm[:], io["enc_b1"][m * 128:(m + 1) * 128, :])
        t = acc_p.tile([128, T], F32R, tag=f"acc_{m}")
        act.activation(t[:], p[:], AF.Prelu, bias=ebm[:], alpha=a_enc1)
        e1T.append(t)

    ew2 = [w512.tile([128, D_MODEL], F32R, tag="w512", name=f"ew2_{k}") for k in range(4)]
    for k in range(4):
        sync.dma_start(ew2[k][:], _f32r(io["enc_w2"][k * 128:(k + 1) * 128, :]))
    eb2r = rowb.tile([1, D_MODEL], F32R, tag="rowbias")
    sync.dma_start(eb2r[:], _f32r(io["enc_b2r"][:]))
    x = []
    for m in range(NT):
        p = psA.tile([128, D_MODEL], F32, tag="pA")
        for k in range(4):
            pe.matmul(p[:], e1T[k][:, m * 128:(m + 1) * 128], ew2[k][:],
                      start=(k == 0), stop=False)
        pe.matmul(p[:], ones_r[:], eb2r[:], start=False, stop=True)
        xt = persist.tile([128, D_MODEL], F32, tag=f"x_{m}")
        act.activation(xt[:], p[:], AF.Prelu, alpha=a_enc2)
        x.append(xt)

    # ---------------- relative position embeddings ----------------
    posT = wonce.tile([1, 2 * T], F32R, tag="posT")
    sync.dma_start(posT[:], _f32r(io["posT"][:]))
    rw1 = wonce.tile([1, D_HEAD], F32R, tag="rw1")
    sync.dma_start(rw1[:], _f32r(io["rp_w1"][:]))
    rb1 = wonce.tile([64, 1], F32, tag="rb1")
    sync.dma_start(rb1[:], io["rp_b1"][:])
    rw2 = wonce.tile([64, D_HEAD], F32R, tag="rw2")
    sync.dma_start(rw2[:], _f32r(io["rp_w2"][:]))
    rb2 = wonce.tile([64, 1], F32, tag="rb2")
    sync.dma_start(rb2[:], io["rp_b2"][:])

    r1T = wonce.tile([64, 2 * T], F32R, tag="r1T")
    rT_pad = persist.tile([128, RPAD], BF16, tag="rT_pad")
    for c0 in (0, 512):
        p = psS.tile([64, 512], F32, tag="pS")
        pe.matmul(p[:], rw1[:], posT[:, c0:c0 + 512], start=True, stop=True)
        act.activation(r1T[:, c0:c0 + 512], p[:], AF.Prelu, bias=rb1[:], alpha=a_rp)
    for c0 in (0, 512):
        p = psS.tile([64, 512], F32, tag="pS")
        pe.matmul(p[:], rw2[:], r1T[:, c0:c0 + 512], start=True, stop=True)
        dve.tensor_scalar(rT_pad[0:64, c0:c0 + 512], p[:], rb2[:], INV_SQRT_D,
                          op0=ALU.add, op1=ALU.mult)
    dve.tensor_scalar(rT_pad[0:64, 1023:RPAD], rT_pad[0:64, 0:RPAD - 1023], 0.0, None,
                      op0=ALU.mult)
    dve.tensor_copy(rT_pad[64:128, :], rT_pad[0:64, :])

    def dump_x(x):
        for m in range(NT):
            sync.dma_start(io["out"][m * 128:(m + 1) * 128, 0:D_OUT],
                           x[m][:, 0:D_OUT])
            sync.dma_start(io["out"][m * 128:(m + 1) * 128, D_OUT:148],
                           x[m][:, D_OUT:148])
    if stages == "enc":
        dump_x(x)
        return

    # ---------------- layers ----------------
    def layernorm_to_T(xtiles, g_dram, b_dram):
        gcol = [colb.tile([128, 1], F32, tag="colbias", name=f"gcol{k}") for k in range(4)]
        bcol = [colb.tile([128, 1], F32, tag="colbias", name=f"bcolv{k}") for k in range(4)]
        for k in range(4):
            sync.dma_start(gcol[k][:], g_dram[k * 128:(k + 1) * 128, :])
            sync.dma_start(bcol[k][:], b_dram[k * 128:(k + 1) * 128, :])
        hT = [hT_p.tile([128, T], F32R, tag=f"hT_{k}", name=f"hT_{k}") for k in range(4)]
        for m in range(NT):
            xt = xtiles[m]
            s = st_p.tile([128, 1], F32, tag="ln_s")
            dve.tensor_reduce(s[:], xt[:], axis=AX.X, op=ALU.add)
            scr = ln_p.tile([128, D_MODEL], BF16, tag="ln_scr")
            ss = st_p.tile([128, 1], F32, tag="ln_ss")
            act.activation(scr[:], xt[:], AF.Square, accum_out=ss[:])
            mean = st_p.tile([128, 1], F32, tag="ln_m")
            dve.tensor_scalar(mean[:], s[:], 1.0 / D_MODEL, None, op0=ALU.mult)
            msq = st_p.tile([128, 1], F32, tag="ln_msq")
            dve.tensor_tensor(msq[:], mean[:], mean[:], op=ALU.mult)
            var = st_p.tile([128, 1], F32, tag="ln_v")
            dve.scalar_tensor_tensor(var[:], ss[:], 1.0 / D_MODEL, msq[:],
                                     op0=ALU.mult, op1=ALU.subtract)
            std = st_p.tile([128, 1], F32, tag="ln_std")
            import os as _os
            if _os.environ.get("KERNEL_NO_SQRT"):
                act.activation(std[:], var[:], AF.Square, bias=eps_t[:])
            else:
                act.activation(std[:], var[:], AF.Sqrt, bias=eps_t[:])
            rstd = st_p.tile([128, 1], F32, tag="ln_r")
            dve.reciprocal(rstd[:], std[:])
            t1 = ln_p.tile([128, D_MODEL], F32R, tag="ln_t1")
            dve.tensor_scalar(t1[:], xt[:], mean[:], rstd[:],
                              op0=ALU.subtract, op1=ALU.mult)
            for k in range(4):
                p = psA.tile([128, 128], F32, tag="pA")
                pe.transpose(_f32r(p[:]), t1[:, k * 128:(k + 1) * 128], id_r[:])
                dve.tensor_scalar(hT[k][:, m * 128:(m + 1) * 128], p[:],
                                  gcol[k][:], bcol[k][:], op0=ALU.mult, op1=ALU.add)
        return hT

    n_layer_run = N_LAYER if stages in ("full",) else 1
    for li in range(n_layer_run):
        # ===== attention =====
        hT = layernorm_to_T(x, io["att_ln_g"][li], io["att_ln_b"][li])

        qT, kT, vbf = [], [], []
        wq_cur = None
        for m in range(12):
            if m % 4 == 0:
                g = m // 4
                wq_cur = [wq_p.tile([128, 512], F32R, tag="wqkv", name=f"wq{m}_{k}") for k in range(4)]
                for k in range(4):
                    sync.dma_start(wq_cur[k][:], _f32r(
                        io["att_qkv_w"][li][k * 128:(k + 1) * 128,
                                            g * 512:(g + 1) * 512]))
            p = psA.tile([128, T], F32, tag="pA")
            for k in range(4):
                pe.matmul(p[:], wq_cur[k][:, (m % 4) * 128:(m % 4 + 1) * 128],
                          hT[k][:], start=(k == 0), stop=(k == 3))
            bcol = colb.tile([128, 1], F32, tag="colbias")
            sync.dma_start(bcol[:], io["att_qkv_b"][li][m * 128:(m + 1) * 128, :])
            if m < 4:       # q -> bf16
                t = qkv_p.tile([128, T], BF16, tag=f"qT_{m}")
                dve.tensor_scalar(t[:], p[:], bcol[:], None, op0=ALU.add)
                qT.append(t)
            elif m < 8:     # k -> bf16, pre-scaled by 1/sqrt(d)
                t = qkv_p.tile([128, T], BF16, tag=f"kT_{m - 4}")
                dve.tensor_scalar(t[:], p[:], bcol[:], INV_SQRT_D,
                                  op0=ALU.add, op1=ALU.mult)
                kT.append(t)
            else:           # v -> bf16
                t = qkv_p.tile([128, T], BF16, tag=f"vbf_{m - 8}")
                dve.tensor_scalar(t[:], p[:], bcol[:], None, op0=ALU.add)
                vbf.append(t)

        v_jd = [qkv_p.tile([128, N_HEAD * D_HEAD], BF16, tag=f"vjd_{jb}", name=f"vjd_{jb}")
                for jb in range(NT)]
        for h in range(N_HEAD):
            vt = vbf[h // 2]
            p0 = (h % 2) * 64
            for jb in range(NT):
                pv = psTR.tile([128, 64], BF16, tag="pTR")
                pe.transpose(pv[:], vt[p0:p0 + 64, jb * 128:(jb + 1) * 128],
                             id_b[p0:p0 + 64, p0:p0 + 64])
                dve.tensor_copy(v_jd[jb][:, h * 64:(h + 1) * 64], pv[:])

        oT = [oT_p.tile([128, T], BF16, tag=f"oT_{t}", name=f"oT_{t}") for t in range(NT)]
        for h in range(N_HEAD):
            p0 = (h % 2) * 64
            for ti in range(NT):
                qs = qT[h // 2][p0:p0 + 64, ti * 128:(ti + 1) * 128]
                pac = psA.tile([128, T], F32, tag="pA")
                pe.matmul(pac[:], qs, kT[h // 2][p0:p0 + 64, :], start=True, stop=True)
                w0 = 384 - 128 * ti
                pbd = psBD.tile([128, RW], F32, tag="pBD")
                pe.matmul(pbd[:, 0:512], qs, rT_pad[p0:p0 + 64, w0:w0 + 512],
                          start=True, stop=True)
                pe.matmul(pbd[:, 512:RW], qs, rT_pad[p0:p0 + 64, w0 + 512:w0 + RW],
                          start=True, stop=True)
                bd_sb = atr.tile([128, RW], BF16, tag="bd_sb")
                dve.tensor_copy(bd_sb[:], pbd[:])
                shift = atr.tile([128, T], BF16, tag="shift")
                diag = bass.AP(tensor=bd_sb[:].tensor,
                               offset=bd_sb[:].offset + 127,
                               ap=[[RW - 1, 128], [1, T]])
                act.dma_start(shift[:], diag)
                scores = atr.tile([128, T], BF16, tag="scores")
                dve.tensor_tensor(scores[:], pac[:], shift[:], op=ALU.add)
                expb = atr.tile([128, T], BF16, tag="expb")
                sums = st_p.tile([128, 1], F32, tag="sums")
                act.activation(expb[:], scores[:], AF.Exp, accum_out=sums[:])
                recip = st_p.tile([128, 1], F32, tag="recip")
                dve.reciprocal(recip[:], sums[:])
                attn = atr.tile([128, T], BF16, tag="attn")
                dve.tensor_scalar(attn[:], expb[:], recip[:], None, op0=ALU.mult)
                pat = psTR.tile([128, 512], BF16, tag="pTR")
                for jb in range(NT):
                    pe.transpose(pat[:, jb * 128:(jb + 1) * 128],
                                 attn[:, jb * 128:(jb + 1) * 128], id_b[:])
                attnT = atr.tile([128, T], BF16, tag="attnT")
                act.activation(attnT[:], pat[:], AF.Copy)
                po = psS.tile([64, 128], F32, tag="pS")
                for jb in range(NT):
                    pe.matmul(po[:], v_jd[jb][:, h * 64:(h + 1) * 64],
                              attnT[:, jb * 128:(jb + 1) * 128],
                              start=(jb == 0), stop=(jb == 3))
                dve.tensor_copy(oT[h // 2][p0:p0 + 64, ti * 128:(ti + 1) * 128],
                                po[:])

        wo = [w512.tile([128, D_MODEL], BF16, tag="w512b", name=f"wo_{k}") for k in range(4)]
        for k in range(4):
            sync.dma_start(wo[k][:], io["att_o_wb"][li][k * 128:(k + 1) * 128, :])
        obr = rowb.tile([1, D_MODEL], F32R, tag="rowbias")
        sync.dma_start(obr[:], _f32r(io["att_o_br"][li]))
        for m in range(NT):
            p = psA.tile([128, D_MODEL], F32, tag="pA")
            for k in range(4):
                pe.matmul(p[:], oT[k][:, m * 128:(m + 1) * 128], wo[k][:],
                          start=(k == 0), stop=False)
            pe.matmul(p[:], ones_r[:], obr[:], start=False, stop=True)
            dve.tensor_tensor(x[m][:], p[:], x[m][:], op=ALU.add)

        if stages == "att1":
            dump_x(x)
            return
        # ===== feed-forward =====
        h2T = layernorm_to_T(x, io["pff_ln_g"][li], io["pff_ln_b"][li])
        a1T = []
        wf_cur = None
        for m in range(16):
            if m % 4 == 0:
                g = m // 4
                wf_cur = [wf1_p.tile([128, 512], F32R, tag="wff1", name=f"wf{m}_{k}") for k in range(4)]
                for k in range(4):
                    sync.dma_start(wf_cur[k][:], _f32r(
                        io["pff_w1"][li][k * 128:(k + 1) * 128,
                                         g * 512:(g + 1) * 512]))
            p = psA.tile([128, T], F32, tag="pA")
            for k in range(4):
                pe.matmul(p[:], wf_cur[k][:, (m % 4) * 128:(m % 4 + 1) * 128],
                          h2T[k][:], start=(k == 0), stop=(k == 3))
            bcol = colb.tile([128, 1], F32, tag="colbias")
            sync.dma_start(bcol[:], io["pff_b1"][li][m * 128:(m + 1) * 128, :])
            t = a1_p.tile([128, T], BF16, tag=f"a1T_{m}")
            act.activation(t[:], p[:], AF.Relu, bias=bcol[:])
            a1T.append(t)
        wf2 = [wf2_p.tile([128, D_MODEL], BF16, tag="wff2", name=f"wf2_{k}") for k in range(16)]
        for k in range(16):
            sync.dma_start(wf2[k][:], io["pff_w2b"][li][k * 128:(k + 1) * 128, :])
        fbr = rowb.tile([1, D_MODEL], F32R, tag="rowbias")
        sync.dma_start(fbr[:], _f32r(io["pff_b2r"][li]))
        for m in range(NT):
            p = psA.tile([128, D_MODEL], F32, tag="pA")
            for k in range(16):
                pe.matmul(p[:], a1T[k][:, m * 128:(m + 1) * 128], wf2[k][:],
                          start=(k == 0), stop=False)
            pe.matmul(p[:], ones_r[:], fbr[:], start=False, stop=True)
            dve.tensor_tensor(x[m][:], p[:], x[m][:], op=ALU.add)

    if stages == "ffn1":
        dump_x(x)
        return
    # ---------------- final layernorm ----------------
    xfT = layernorm_to_T(x, io["final_ln_g"], io["final_ln_b"])

    # ---------------- phase decoder ----------------
    wp1 = [w512.tile([128, D_DEC], F32R, tag="w512", name=f"wp1_{k}") for k in range(4)]
    for k in range(4):
        sync.dma_start(wp1[k][:], _f32r(io["pd_w1"][k * 128:(k + 1) * 128, :]))
    p1T = []
    for m in range(NT):
        p = psA.tile([128, T], F32, tag="pA")
        for k in range(4):
            pe.matmul(p[:], wp1[k][:, m * 128:(m + 1) * 128], xfT[k][:],
                      start=(k == 0), stop=(k == 3))
        bcol = colb.tile([128, 1], F32, tag="colbias")
        sync.dma_start(bcol[:], io["pd_b1"][m * 128:(m + 1) * 128, :])
        t = zn_p.tile([128, T], F32R, tag=f"znT_{m}", name=f"p1T_{m}")
        act.activation(t[:], p[:], AF.Prelu, bias=bcol[:], alpha=a_pd)
        p1T.append(t)
    wp2 = [w512.tile([128, D_PHASE], F32R, tag="w512", name=f"wp2_{k}") for k in range(4)]
    for k in range(4):
        sync.dma_start(wp2[k][:], _f32r(io["pd_w2"][k * 128:(k + 1) * 128, :]))
    pb2r = rowb.tile([1, D_PHASE], F32R, tag="pb2r")
    sync.dma_start(pb2r[:], _f32r(io["pd_b2r"][:]))
    ph = []
    phT = persist.tile([16, T], F32R, tag="phT")
    for m in range(NT):
        p = psS.tile([128, 16], F32, tag="pS")
        for k in range(4):
            pe.matmul(p[:], p1T[k][:, m * 128:(m + 1) * 128], wp2[k][:],
                      start=(k == 0), stop=False)
        pe.matmul(p[:], ones_r[:], pb2r[:], start=False, stop=True)
        pht = persist.tile([128, D_PHASE], F32R, tag=f"ph_{m}")
        dve.tensor_tensor(pht[:], p[:], phase_sb[m][:].bitcast(F32), op=ALU.add)
        ph.append(pht)
        sync.dma_start(io["out"][m * 128:(m + 1) * 128, D_OUT:D_OUT + D_PHASE],
                       pht[:].bitcast(F32))
        pt = psS.tile([16, 128], F32, tag="pS")
        pe.transpose(_f32r(pt[:]), pht[:], id_r[:])
        dve.tensor_copy(phT[:, m * 128:(m + 1) * 128], pt[:])

    # ---------------- gating network ----------------
    gw1 = wonce.tile([16, D_GATE], F32R, tag="gw1")
    sync.dma_start(gw1[:], _f32r(io["g_w1"][:]))
    gb1 = colb.tile([128, 1], F32, tag="colbias")
    sync.dma_start(gb1[:], io["g_b1"][:])
    pg = psA.tile([128, T], F32, tag="pA")
    pe.matmul(pg[:], gw1[:], phT[:], start=True, stop=True)
    g1T = misc.tile([128, T], F32R, tag="g1T")
    act.activation(g1T[:], pg[:], AF.Prelu, bias=gb1[:], alpha=a_g1)

    gw2 = wonce.tile([128, D_GATE], F32R, tag="gw2")
    sync.dma_start(gw2[:], _f32r(io["g_w2"][:]))
    gb2 = colb.tile([128, 1], F32, tag="colbias")
    sync.dma_start(gb2[:], io["g_b2"][:])
    pg2 = psA.tile([128, T], F32, tag="pA")
    pe.matmul(pg2[:], gw2[:], g1T[:], start=True, stop=True)
    g2T = misc.tile([128, T], F32R, tag="g2T")
    act.activation(g2T[:], pg2[:], AF.Prelu, bias=gb2[:], alpha=a_g2)

    gw3 = wonce.tile([128, E], F32R, tag="gw3")
    sync.dma_start(gw3[:], _f32r(io["g_w3"][:]))
    gb3 = wonce.tile([E, 1], F32, tag="gb3")
    sync.dma_start(gb3[:], io["g_b3"][:])
    pg3 = psS.tile([E, T], F32, tag="pS")
    pe.matmul(pg3[:], gw3[:], g2T[:], start=True, stop=True)
    lT = misc.tile([E, T], F32R, tag="lT")
    dve.tensor_scalar(lT[:], pg3[:], gb3[:], None, op0=ALU.add)

    gw = []
    gwT = persist.tile([E, T], F32R, tag="gwT")
    for m in range(NT):
        p = psS.tile([128, E], F32, tag="pS")
        pe.transpose(_f32r(p[:]), lT[:, m * 128:(m + 1) * 128], id_r[0:E, 0:E])
        ge = misc.tile([128, E], F32, tag="ge")
        gs = st_p.tile([128, 1], F32, tag="gs")
        act.activation(ge[:], p[:], AF.Exp, accum_out=gs[:])
        gr = st_p.tile([128, 1], F32, tag="gr")
        dve.reciprocal(gr[:], gs[:])
        gwm = persist.tile([128, E], F32R, tag=f"gw_{m}")
        dve.tensor_scalar(gwm[:], ge[:], gr[:], None, op0=ALU.mult)
        gw.append(gwm)
        pt = psS.tile([E, 128], F32, tag="pS")
        pe.transpose(_f32r(pt[:]), gwm[:], id_r[:])
        dve.tensor_copy(gwT[:, m * 128:(m + 1) * 128], pt[:])

    # ---------------- MoE blends ----------------
    zT = xfT
    for L, (wname, bname, alpha, NOUT) in enumerate((
            ("ml_w0", "ml_b0", a_ml0, D_DEC),
            ("ml_w1", "ml_b1", a_ml1, D_DEC),
            ("ml_w2p", "ml_b2p", None, D_OUT_PAD))):
        mlb = wonce.tile([E, NOUT], F32R, tag=f"mlb_{L}")
        sync.dma_start(mlb[:], _f32r(io[bname][:]))
        acc = [acc_p.tile([128, NOUT], F32R, tag=f"acc_{m}", name=f"accL{L}_{m}") for m in range(NT)]
        for m in range(NT):
            p = psA.tile([128, NOUT], F32, tag="pA")
            pe.matmul(p[:], gwT[:, m * 128:(m + 1) * 128], mlb[:],
                      start=True, stop=True)
            dve.tensor_copy(acc[m][:], p[:])
        for e in range(E):
            we = [wml_p.tile([128, NOUT], F32R, tag="wml", name=f"we{L}_{e}_{k}") for k in range(4)]
            for k in range(4):
                sync.dma_start(we[k][:], _f32r(io[wname][e][k * 128:(k + 1) * 128, :]))
            for m in range(NT):
                p = psA.tile([128, NOUT], F32, tag="pA")
                for k in range(4):
                    pe.matmul(p[:], zT[k][:, m * 128:(m + 1) * 128], we[k][:],
                              start=(k == 0), stop=(k == 3))
                dve.scalar_tensor_tensor(acc[m][:], p[:],
                                         gw[m][:, e:e + 1].bitcast(F32),
                                         acc[m][:].bitcast(F32),
                                         op0=ALU.mult, op1=ALU.add)
        if L < 2:
            znT = [zn_p.tile([128, T], F32R, tag=f"znT_{k}", name=f"znT{L}_{k}") for k in range(4)]
            for m in range(NT):
                for k in range(4):
                    p = psA.tile([128, 128], F32, tag="pA")
                    pe.transpose(_f32r(p[:]), acc[m][:, k * 128:(k + 1) * 128],
                                 id_r[:])
                    act.activation(znT[k][:, m * 128:(m + 1) * 128], p[:],
                                   AF.Prelu, alpha=alpha)
            zT = znT
        else:
            for m in range(NT):
                sync.dma_start(io["out"][m * 128:(m + 1) * 128, 0:D_OUT],
                               acc[m][:, 0:D_OUT].bitcast(F32))


def build_program(alphas, n_reps=1, stages="full"):
    nc = bacc.Bacc("TRN2", target_bir_lowering=False, debug=False)
    io = {}

    def inp(name, shape, dt=F32):
        io[name] = nc.dram_tensor(name, list(shape), dt, kind="ExternalInput").ap()

    inp("motion", (T, D_MOTION))
    inp("phase", (T, D_PHASE))
    inp("dm_row", (1, T))
    inp("posT", (1, 2 * T))
    inp("ones_row", (1, 128))
    inp("id128", (128, 128))
    inp("id128b", (128, 128), BF16)

    inp("enc_w1", (D_IN, D_ENC)); inp("enc_b1", (D_ENC, 1))
    inp("enc_w2", (D_ENC, D_MODEL)); inp("enc_b2r", (1, D_MODEL))
    inp("rp_w1", (1, D_HEAD)); inp("rp_b1", (D_HEAD, 1))
    inp("rp_w2", (D_HEAD, D_HEAD)); inp("rp_b2", (D_HEAD, 1))
    inp("att_qkv_w", (N_LAYER, D_MODEL, 3 * D_MODEL))
    inp("att_qkv_b", (N_LAYER, 3 * D_MODEL, 1))
    inp("att_o_wb", (N_LAYER, D_MODEL, D_MODEL), BF16)
    inp("att_o_br", (N_LAYER, 1, D_MODEL))
    inp("att_ln_g", (N_LAYER, D_MODEL, 1)); inp("att_ln_b", (N_LAYER, D_MODEL, 1))
    inp("pff_w1", (N_LAYER, D_MODEL, D_PFF)); inp("pff_b1", (N_LAYER, D_PFF, 1))
    inp("pff_w2b", (N_LAYER, D_PFF, D_MODEL), BF16)
    inp("pff_b2r", (N_LAYER, 1, D_MODEL))
    inp("pff_ln_g", (N_LAYER, D_MODEL, 1)); inp("pff_ln_b", (N_LAYER, D_MODEL, 1))
    inp("final_ln_g", (D_MODEL, 1)); inp("final_ln_b", (D_MODEL, 1))
    inp("pd_w1", (D_MODEL, D_DEC)); inp("pd_b1", (D_DEC, 1))
    inp("pd_w2", (D_DEC, D_PHASE)); inp("pd_b2r", (1, D_PHASE))
    inp("g_w1", (D_PHASE, D_GATE)); inp("g_b1", (D_GATE, 1))
    inp("g_w2", (D_GATE, D_GATE)); inp("g_b2", (D_GATE, 1))
    inp("g_w3", (D_GATE, E)); inp("g_b3", (E, 1))
    inp("ml_w0", (E, D_MODEL, D_DEC)); inp("ml_b0", (E, D_DEC))
    inp("ml_w1", (E, D_DEC, D_DEC)); inp("ml_b1", (E, D_DEC))
    inp("ml_w2p", (E, D_DEC, D_OUT_PAD)); inp("ml_b2p", (E, D_OUT_PAD))

    io["out"] = nc.dram_tensor("out", [T, D_OUT + D_PHASE], F32,
                               kind="ExternalOutput").ap()

    with tile.TileContext(nc) as tc, ExitStack() as ctx:
        if n_reps == 1:
            build_forward(nc, tc, ctx, alphas, io, stages)
        else:
            with tc.For_i(0, n_reps, 1):
                with ExitStack() as inner:
                    build_forward(nc, tc, inner, alphas, io, stages)
    nc.compile()
    return nc


def prepare_weight_maps(params):
    import ml_dtypes
    p = {k: np.asarray(v) for k, v in params.items()}
    f = np.float32
    w = {}
    w["enc_w1"] = p["enc_w1"].astype(f)
    w["enc_b1"] = p["enc_b1"].astype(f).reshape(-1, 1)
    w["enc_w2"] = p["enc_w2"].astype(f)
    w["enc_b2r"] = p["enc_b2"].astype(f).reshape(1, -1)
    w["rp_w1"] = p["rp_w1"].astype(f).reshape(1, D_HEAD)
    w["rp_b1"] = p["rp_b1"].astype(f).reshape(-1, 1)
    w["rp_w2"] = p["rp_w2"].astype(f)
    w["rp_b2"] = p["rp_b2"].astype(f).reshape(-1, 1)
    w["att_qkv_w"] = p["att_qkv_w"].astype(f)
    w["att_qkv_b"] = p["att_qkv_b"].astype(f).reshape(N_LAYER, -1, 1)
    w["att_o_wb"] = p["att_o_w"].astype(ml_dtypes.bfloat16)
    w["att_o_br"] = p["att_o_b"].astype(f).reshape(N_LAYER, 1, -1)
    w["att_ln_g"] = p["att_ln_g"].astype(f).reshape(N_LAYER, -1, 1)
    w["att_ln_b"] = p["att_ln_b"].astype(f).reshape(N_LAYER, -1, 1)
    w["pff_w1"] = p["pff_w1"].astype(f)
    w["pff_b1"] = p["pff_b1"].astype(f).reshape(N_LAYER, -1, 1)
    w["pff_w2b"] = p["pff_w2"].astype(ml_dtypes.bfloat16)
    w["pff_b2r"] = p["pff_b2"].astype(f).reshape(N_LAYER, 1, -1)
    w["pff_ln_g"] = p["pff_ln_g"].astype(f).reshape(N_LAYER, -1, 1)
    w["pff_ln_b"] = p["pff_ln_b"].astype(f).reshape(N_LAYER, -1, 1)
    w["final_ln_g"] = p["final_ln_g"].astype(f).reshape(-1, 1)
    w["final_ln_b"] = p["final_ln_b"].astype(f).reshape(-1, 1)
    w["pd_w1"] = p["pd_w1"].astype(f)
    w["pd_b1"] = p["pd_b1"].astype(f).reshape(-1, 1)
    w["pd_w2"] = p["pd_w2"].astype(f)
    w["pd_b2r"] = p["pd_b2"].astype(f).reshape(1, -1)
    w["g_w1"] = p["g_w1"].astype(f)
    w["g_b1"] = p["g_b1"].astype(f).reshape(-1, 1)
    w["g_w2"] = p["g_w2"].astype(f)
    w["g_b2"] = p["g_b2"].astype(f).reshape(-1, 1)
    w["g_w3"] = p["g_w3"].astype(f)
    w["g_b3"] = p["g_b3"].astype(f).reshape(-1, 1)
    w["ml_w0"] = p["ml_w0"].astype(f)
    w["ml_b0"] = p["ml_b0"].astype(f)
    w["ml_w1"] = p["ml_w1"].astype(f)
    w["ml_b1"] = p["ml_b1"].astype(f)
    w2p = np.zeros((E, D_DEC, D_OUT_PAD), f)
    w2p[:, :, :D_OUT] = p["ml_w2"].astype(f)
    w["ml_w2p"] = w2p
    b2p = np.zeros((E, D_OUT_PAD), f)
    b2p[:, :D_OUT] = p["ml_b2"].astype(f)
    w["ml_b2p"] = b2p
    return w


def extract_alphas(params):
    return tuple(float(np.asarray(params[k])) for k in
                 ("enc_a1", "enc_a2", "rp_a", "pd_a", "g_a1", "g_a2",
                  "ml_a0", "ml_a1"))


_CACHE = {}


def get_program(alphas, n_reps=1, stages="full"):
    key = (alphas, n_reps, stages)
    if key not in _CACHE:
        _CACHE[key] = build_program(alphas, n_reps, stages)
    return _CACHE[key]


def make_in_maps(motion, phase, midway_targets, params):
    import ml_dtypes
    motion = np.asarray(motion, np.float32)
    phase = np.asarray(phase, np.float32)
    mt = np.asarray(midway_targets).astype(np.int64)

    dm = np.zeros(T, np.float32)
    dm[:CTX] = 1.0
    dm[-1] = 1.0
    dm[mt] = 1.0

    shared = prepare_weight_maps(params)
    shared["dm_row"] = dm.reshape(1, T)
    shared["posT"] = np.arange(-T + 1, T + 1, dtype=np.float32).reshape(1, 2 * T)
    shared["ones_row"] = np.ones((1, 128), np.float32)
    shared["id128"] = np.eye(128, dtype=np.float32)
    shared["id128b"] = np.eye(128, dtype=ml_dtypes.bfloat16)

    in_maps = []
    for c in range(N_CORES):
        m = dict(shared)
        m["motion"] = motion[c]
        m["phase"] = phase[c]
        in_maps.append(m)
    return in_maps


def kernel(motion, phase, midway_targets, params):
    alphas = extract_alphas(params)
    nc = get_program(alphas, 1)
    in_maps = make_in_maps(motion, phase, midway_targets, params)
    res = run_bass_kernel_spmd(nc, in_maps, list(range(N_CORES)))
    out = np.stack([res.results[c]["out"] for c in range(N_CORES)], axis=0)
    return out.astype(np.float32)
